# revision 1
# baseline (speedup 1.0000x reference)
"""Trainium2 Bass kernel for nn_Attention (dense transformer spatial attention).

Reference computation (per batch b):
    q = Wq @ x   (1x1 conv over channels), k = Wk @ c, v = Wv @ c
    per head h (8 heads, head_dim 32, n = 64*64 = 4096 tokens):
        S = (q_h^T k_h) * DIM**-0.5 ; P = softmax(S, axis=-1) ; o_h = v_h P^T
    out = Wo @ concat(o_h)

Sharding (8 cores): core c handles batch b = c//2 and heads 4*(c%2) .. +4
(tensor-parallel over heads via weight row/col slicing).  The two cores of a
batch produce partial outputs Y = Wo_slice @ o_slice which the host sums.

Per-core dataflow (everything fp32-accumulated, fp16 operands for the big
matmuls; scale folded into Wq host-side):
  - Q/K/V projections: fp32 matmuls, PSUM->SBUF evacuation casts to fp16.
    q_sb/k_sb layout: (128 = 4 heads x 32 dims, 4096 tokens).
  - v is PE-transposed to vT (token-major) with a ones column appended, so the
    AV matmul also produces the softmax denominator l = sum_j exp(s).
  - Scores are computed *transposed*: S^T[j, i] = sum_d k[d,j] q[d,i] using
    4x row-tiled matmuls (K = 32 per head, 2 heads per pass).  exp() runs on
    ScalarE directly from PSUM (no max subtraction: |scores| <~ 2 by
    construction), output fp16 in SBUF.
  - AV: col-tiled matmuls (M = 33: 32 dims + ones row) accumulate
    outT[(h,d), i] over all j in PSUM, already in the layout the output
    projection consumes.  Normalization by 1/l happens on evacuation.
  - Output projection: fp16 matmul with WoT, fp32 result DMA'd out.
"""

import os
import sys

import numpy as np

for _p in ("/opt/trn_rl_repo", "/root/.axon_site/_ro/trn_rl_repo"):
    if os.path.isdir(_p) and _p not in sys.path:
        sys.path.insert(0, _p)

import concourse.bass as bass
import concourse.tile as tile
from concourse import bacc, mybir
from concourse.bass import ts
from concourse.bass_utils import run_bass_kernel_spmd
from concourse.masks import make_identity

DIM = 512
HEAD = 8
ATTN_DIM = 256
HEAD_DIM = 32
N = 4096  # 64 * 64 tokens
SCALE = DIM ** -0.5

N_CORES = 8
HEADS_PER_CORE = 4  # 4 heads x 32 dims = 128 partitions
NI = 512   # i-tile (query tokens per score matmul rhs)
NJ = 128   # j-tile (key tokens per score matmul lhsT)
JBLK = 6   # j-tiles per mode block (row-tiled scores vs col-tiled AV)

F32 = mybir.dt.float32
F16 = mybir.dt.float16


def build_nc():
    nc = bacc.Bacc()

    x_d = nc.dram_tensor("x", [DIM, N], F16, kind="ExternalInput").ap()
    c_d = nc.dram_tensor("c", [DIM, N], F16, kind="ExternalInput").ap()
    wqt_d = nc.dram_tensor("wqt", [DIM, 128], F16, kind="ExternalInput").ap()
    wkt_d = nc.dram_tensor("wkt", [DIM, 128], F16, kind="ExternalInput").ap()
    wvt_d = nc.dram_tensor("wvt", [DIM, 128], F16, kind="ExternalInput").ap()
    wot_d = nc.dram_tensor("wot", [128, DIM], F16, kind="ExternalInput").ap()
    y_d = nc.dram_tensor("y", [DIM, N], F32, kind="ExternalOutput").ap()

    from contextlib import ExitStack

    with tile.TileContext(nc) as tc, ExitStack() as stk:
        persist = stk.enter_context(tc.tile_pool(name="persist", bufs=1))

        q_sb = persist.tile([128, N], F16)
        k_sb = persist.tile([128, N], F16)
        # vT: (token-in-chunk, j_chunk, head, 32 dims + ones col)
        vT_sb = persist.tile([128, N // NJ, HEADS_PER_CORE, HEAD_DIM + 1], F16)
        wot_sb = persist.tile([128, DIM], F16)
        ident = persist.tile([128, 128], F16)

        nc.sync.dma_start(out=wot_sb, in_=wot_d)
        make_identity(nc, ident)
        nc.vector.memset(vT_sb[:, :, :, HEAD_DIM:], 1.0)

        # Preload the exp activation table set during the DMA lead-in so the
        # first real exp doesn't pay the ~2.7us ACT_TABLE_LOAD.
        warm_sb = persist.tile([1, 32], F32)
        nc.vector.memset(warm_sb, 0.0)
        nc.scalar.activation(out=warm_sb, in_=warm_sb,
                             func=mybir.ActivationFunctionType.Exp)

        # ---------------- Phase 1: K/Q projections ----------------
        # c/w/v stay alive into the attention phase (V projection is emitted
        # inside the first attention block so scores/exp start ASAP).
        cw_pool = stk.enter_context(tc.tile_pool(name="cw", bufs=1))

        w_sb = {}
        for nm, d in (("wkt", wkt_d), ("wqt", wqt_d), ("wvt", wvt_d)):
            w = cw_pool.tile([128, 4, 128], F16, tag=nm)
            nc.sync.dma_start(out=w, in_=d.rearrange("(c p) m -> p c m", p=128))
            w_sb[nm] = w
        c_t = []
        for cc in range(4):
            t = cw_pool.tile([128, N], F16, tag="c_in", bufs=4)
            nc.sync.dma_start(out=t, in_=c_d[ts(cc, 128), :])
            c_t.append(t)
        v_sb = cw_pool.tile([128, N], F16, tag="v_sb")

        def project(psum_pool, tag, wname, src, dst):
            w = w_sb[wname]
            for t in range(N // NI):
                ps = psum_pool.tile([128, NI], F32, tag=tag)
                for cc in range(4):
                    nc.tensor.matmul(
                        ps, lhsT=w[:, cc, :], rhs=src[cc][:, ts(t, NI)],
                        start=(cc == 0), stop=(cc == 3),
                    )
                nc.vector.tensor_copy(out=dst[:, ts(t, NI)], in_=ps)

        with tc.tile_pool(name="x_in", bufs=1) as x_pool, \
             tc.tile_pool(name="pj_ps", bufs=2, space="PSUM") as pj_ps:
            # x comes in on the gpsimd DMA queue, concurrent with c on sync
            x_t = []
            for cc in range(4):
                t = x_pool.tile([128, N], F16, tag="x_in", bufs=4)
                nc.gpsimd.dma_start(out=t, in_=x_d[ts(cc, 128), :])
                x_t.append(t)

            project(pj_ps, "pj", "wkt", c_t, k_sb)
            project(pj_ps, "pj", "wqt", x_t, q_sb)

        def emit_v_projection(psum_pool, part, nparts):
            # one slice of the V projection + transposes, interleaved between
            # attention blocks of (i=0, p=0) so ScalarE never starves long
            w = w_sb["wvt"]
            nt = N // NI
            for t in range(part * nt // nparts, (part + 1) * nt // nparts):
                ps = psum_pool.tile([128, NI], F32, tag="av",
                                    name=f"vps_{t}")
                for cc in range(4):
                    nc.tensor.matmul(
                        ps, lhsT=w[:, cc, :], rhs=c_t[cc][:, ts(t, NI)],
                        start=(cc == 0), stop=(cc == 3),
                    )
                nc.vector.tensor_copy(out=v_sb[:, ts(t, NI)], in_=ps)
            nch = N // NJ
            for ch in range(part * nch // nparts, (part + 1) * nch // nparts):
                tp = psum_pool.tile([128, 128], F16, tag="av",
                                    name=f"vtp_{ch}")
                nc.tensor.transpose(tp, v_sb[:, ts(ch, 128)], ident)
                nc.vector.tensor_copy(
                    out=vT_sb[:, ch, :, 0:HEAD_DIM],
                    in_=tp.rearrange("p (h d) -> p h d", h=HEADS_PER_CORE),
                )

        # ---------------- Phase 2: attention ----------------
        # Score PSUM is a stream of 512-wide slots (2 per j-tile: one per
        # head of the active pair), packed 3 per 3-bank psum tile so each
        # exp ACTIVATE covers N=1536 (fewer, cheaper ScalarE instructions).
        SLOTS_PER_TILE = 2
        with tc.tile_pool(name="sc_ps", bufs=2, space="PSUM") as sc_ps, \
             tc.tile_pool(name="av_ps", bufs=2, space="PSUM") as av_ps, \
             tc.tile_pool(name="attn", bufs=1) as at_pool:

            n_i = N // NI   # 8
            n_j = N // NJ   # 32
            n_slots = 2 * n_j  # 64 per (i, p)

            pending_out = []

            def flush_outproj():
                while pending_out:
                    oi, rsb = pending_out.pop(0)
                    for cc in range(4):
                        ps = av_ps.tile([128, NI], F32, tag="av",
                                        name=f"ofps_{oi}_{cc}")
                        nc.tensor.matmul(
                            ps, lhsT=wot_sb[:, ts(cc, 128)], rhs=rsb,
                            start=True, stop=True,
                        )
                        ot = at_pool.tile([128, NI], F32, tag="ot", bufs=4,
                                          name=f"ot_{oi}_{cc}")
                        nc.vector.tensor_copy(out=ot, in_=ps)
                        nc.sync.dma_start(
                            out=y_d[ts(cc, 128), ts(oi, NI)], in_=ot)

            for i in range(n_i):
                raw_sb = at_pool.tile([128, NI], F16, tag="raw", bufs=2)
                for p in range(2):
                    exp_ref = [None] * n_slots  # slot -> (es_tile, offset)
                    state = {"ps": None, "es": None, "n": 0, "uid": 0}

                    def flush_exp():
                        if state["ps"] is not None and state["n"] > 0:
                            width = state["n"] * NI
                            nc.scalar.activation(
                                out=state["es"][:, 0:width],
                                in_=state["ps"][:, 0:width],
                                func=mybir.ActivationFunctionType.Exp,
                            )
                        state["ps"] = None
                        state["n"] = 0

                    def emit_scores(j0, jn):
                        for j in range(j0, j0 + jn):
                            for hh in range(2):
                                if state["ps"] is None:
                                    state["uid"] += 1
                                    u = state["uid"]
                                    state["ps"] = sc_ps.tile(
                                        [128, SLOTS_PER_TILE * NI], F32,
                                        tag="sc", bufs=3,
                                        name=f"scps_{i}_{p}_{u}")
                                    state["es"] = at_pool.tile(
                                        [128, SLOTS_PER_TILE * NI], F16,
                                        tag="exp", bufs=16,
                                        name=f"es_{i}_{p}_{u}")
                                off = state["n"] * NI
                                base = 64 * p + 32 * hh
                                nc.tensor.matmul(
                                    state["ps"][:, off:off + NI],
                                    lhsT=k_sb[base:base + 32, ts(j, NJ)],
                                    rhs=q_sb[base:base + 32, ts(i, NI)],
                                    start=True, stop=True,
                                    tile_position=(base, 0),
                                )
                                exp_ref[2 * j + hh] = (state["es"], off)
                                state["n"] += 1
                                if state["n"] == SLOTS_PER_TILE:
                                    flush_exp()
                        flush_exp()  # block boundary: never hold a partial

                    def emit_av(j0, jn):
                        for j in range(j0, j0 + jn):
                            for hh in range(2):
                                es, off = exp_ref[2 * j + hh]
                                nc.tensor.matmul(
                                    acc[64 * hh:64 * hh + HEAD_DIM + 1, :],
                                    lhsT=vT_sb[:, j, 2 * p + hh, :],
                                    rhs=es[:, off:off + NI],
                                    start=(j == 0), stop=(j == n_j - 1),
                                    tile_position=(0, 64 * hh),
                                    skip_group_check=True,
                                )

                    blocks = []
                    jj = 0
                    while jj < n_j:
                        jn = min(JBLK, n_j - jj)
                        blocks.append((jj, jn))
                        jj += jn

                    emit_scores(*blocks[0])
                    if p == 0:
                        flush_outproj()
                    if i == 0 and p == 0:
                        emit_v_projection(av_ps, 0, 4)
                    acc = av_ps.tile([128, NI], F32, tag="av")
                    for bi in range(1, len(blocks)):
                        emit_scores(*blocks[bi])
                        if i == 0 and p == 0 and bi <= 3:
                            emit_v_projection(av_ps, bi, 4)
                        emit_av(*blocks[bi - 1])
                    emit_av(*blocks[-1])

                    # normalize: raw[32h:32h+32] = acc_head / l_head
                    for hh in range(2):
                        h = 2 * p + hh
                        lr = at_pool.tile([1, NI], F32, tag="lr", bufs=4)
                        rc = at_pool.tile([1, NI], F32, tag="rc", bufs=4)
                        bc = at_pool.tile([32, NI], F32, tag="bc", bufs=4)
                        nc.vector.tensor_copy(
                            out=lr,
                            in_=acc[64 * hh + HEAD_DIM:64 * hh + HEAD_DIM + 1, :],
                        )
                        nc.vector.reciprocal_approx_fast(out=rc, in_=lr)
                        nc.gpsimd.partition_broadcast(bc, rc)
                        nc.vector.tensor_mul(
                            out=raw_sb[ts(h, 32), :],
                            in0=acc[64 * hh:64 * hh + 32, :],
                            in1=bc,
                        )

                pending_out.append((i, raw_sb))
            flush_outproj()

    nc.compile()
    return nc


_NC_CACHE = None


def _get_nc():
    global _NC_CACHE
    if _NC_CACHE is None:
        _NC_CACHE = build_nc()
    return _NC_CACHE


def _shard_inputs(query, context, Wq, Wk, Wv, Wo):
    query = np.asarray(query, dtype=np.float32)
    context = np.asarray(context, dtype=np.float32)
    Wq = np.asarray(Wq, dtype=np.float32)
    Wk = np.asarray(Wk, dtype=np.float32)
    Wv = np.asarray(Wv, dtype=np.float32)
    Wo = np.asarray(Wo, dtype=np.float32)
    b = query.shape[0]
    in_maps = []
    for core in range(N_CORES):
        bb, p = divmod(core, 2)
        sl = slice(128 * p, 128 * (p + 1))
        in_maps.append({
            "x": np.ascontiguousarray(query[bb].reshape(DIM, N).astype(np.float16)),
            "c": np.ascontiguousarray(context[bb].reshape(DIM, N).astype(np.float16)),
            "wqt": np.ascontiguousarray((Wq[sl, :] * SCALE).T.astype(np.float16)),
            "wkt": np.ascontiguousarray(Wk[sl, :].T.astype(np.float16)),
            "wvt": np.ascontiguousarray(Wv[sl, :].T.astype(np.float16)),
            "wot": np.ascontiguousarray(Wo[:, sl].T.astype(np.float16)),
        })
    return in_maps, b


def _run(inputs, trace=False, **kw):
    in_maps, b = _shard_inputs(**inputs)
    nc = _get_nc()
    res = run_bass_kernel_spmd(nc, in_maps, core_ids=list(range(N_CORES)),
                               trace=trace, **kw)
    outs = []
    for bb in range(b):
        y = res.results[2 * bb]["y"] + res.results[2 * bb + 1]["y"]
        outs.append(y.reshape(DIM, 64, 64))
    return np.stack(outs).astype(np.float32), res


def kernel(**inputs):
    out, _ = _run(inputs)
    return out



# revision 8
# speedup vs baseline: 1.4521x; 1.4521x over previous
"""Trainium2 Bass kernel for nn_Attention (dense transformer spatial attention).

Reference computation (per batch b):
    q = Wq @ x   (1x1 conv over channels), k = Wk @ c, v = Wv @ c
    per head h (8 heads, head_dim 32, n = 64*64 = 4096 tokens):
        S = (q_h^T k_h) * DIM**-0.5 ; P = softmax(S, axis=-1) ; o_h = v_h P^T
    out = Wo @ concat(o_h)

Sharding (8 cores): core c handles batch b = c//2 and heads 4*(c%2) .. +4
(tensor-parallel over heads via weight row/col slicing).  The two cores of a
batch produce partial outputs Y = Wo_slice @ o_slice which the host sums.

Per-core dataflow (everything fp32-accumulated, fp16 operands for the big
matmuls; scale folded into Wq host-side):
  - Q/K/V projections: fp32 matmuls, PSUM->SBUF evacuation casts to fp16.
    q_sb/k_sb layout: (128 = 4 heads x 32 dims, 4096 tokens).
  - v is PE-transposed to vT (token-major) with a ones column appended, so the
    AV matmul also produces the softmax denominator l = sum_j exp(s).
  - Scores are computed *transposed*: S^T[j, i] = sum_d k[d,j] q[d,i] using
    4x row-tiled matmuls (K = 32 per head, 2 heads per pass).  exp() runs on
    ScalarE directly from PSUM (no max subtraction: |scores| <~ 2 by
    construction), output fp16 in SBUF.
  - AV: col-tiled matmuls (M = 33: 32 dims + ones row) accumulate
    outT[(h,d), i] over all j in PSUM, already in the layout the output
    projection consumes.  Normalization by 1/l happens on evacuation.
  - Output projection: fp16 matmul with WoT, fp32 result DMA'd out.
"""

import os
import sys

import numpy as np

for _p in ("/opt/trn_rl_repo", "/root/.axon_site/_ro/trn_rl_repo"):
    if os.path.isdir(_p) and _p not in sys.path:
        sys.path.insert(0, _p)

import concourse.bass as bass
import concourse.tile as tile
from concourse import bacc, mybir
from concourse.bass import ts
from concourse.bass_utils import run_bass_kernel_spmd
from concourse.masks import make_identity

DIM = 512
HEAD = 8
ATTN_DIM = 256
HEAD_DIM = 32
N = 4096  # 64 * 64 tokens
SCALE = DIM ** -0.5

N_CORES = 8
HEADS_PER_CORE = 4  # 4 heads x 32 dims = 128 partitions
NI = 512   # i-tile (query tokens per score matmul rhs)
NJ = 128   # j-tile (key tokens per score matmul lhsT)
JBLK = 6   # j-tiles per mode block (row-tiled scores vs col-tiled AV)

F32 = mybir.dt.float32
F16 = mybir.dt.float16
I16 = mybir.dt.int16

# Schraudolph fast-exp on the Vector/GpSimd engines: for fp16, the bit
# pattern of exp(x) is approximately int16(x * 1024/ln2 + 15*1024 + sigma).
# One tensor_scalar (mult, add) with an int16 output view computes it in a
# single instruction; rel err ~N(0, 1.8%) which washes out over the 4096-way
# diffuse softmax (end-to-end absmax rel err ~1e-2 vs the 2e-2 gate).
EXP_A = 1024.0 / float(np.log(2.0))
EXP_B = 15.0 * 1024.0 - 60.0

# Per-(i,p) schedule: which engine runs exp for each of the 16 PSUM score
# tiles (each [128, 2*NI]).  A=ScalarE table exp, D=Vector bit-trick.
# (GpSimd has no PSUM port, so it cannot help with exp.)
EXP_SCHED = "ADADADADADADADAD"


def build_nc():
    nc = bacc.Bacc()

    x_d = nc.dram_tensor("x", [DIM, N], F16, kind="ExternalInput").ap()
    c_d = nc.dram_tensor("c", [DIM, N], F16, kind="ExternalInput").ap()
    wqt_d = nc.dram_tensor("wqt", [DIM, 128], F16, kind="ExternalInput").ap()
    wkt_d = nc.dram_tensor("wkt", [DIM, 128], F16, kind="ExternalInput").ap()
    wvt_d = nc.dram_tensor("wvt", [DIM, 128], F16, kind="ExternalInput").ap()
    wot_d = nc.dram_tensor("wot", [128, DIM], F16, kind="ExternalInput").ap()
    y_d = nc.dram_tensor("y", [DIM, N], F32, kind="ExternalOutput").ap()

    from contextlib import ExitStack

    with tile.TileContext(nc) as tc, ExitStack() as stk:
        persist = stk.enter_context(tc.tile_pool(name="persist", bufs=1))

        q_sb = persist.tile([128, N], F16)
        k_sb = persist.tile([128, N], F16)
        # vT: (token-in-chunk, j_chunk, head, 32 dims + ones col)
        vT_sb = persist.tile([128, N // NJ, HEADS_PER_CORE, HEAD_DIM + 1], F16)
        wot_sb = persist.tile([128, DIM], F16)
        ident = persist.tile([128, 128], F16)

        nc.sync.dma_start(out=wot_sb, in_=wot_d)
        make_identity(nc, ident)
        nc.vector.memset(vT_sb[:, :, :, HEAD_DIM:], 1.0)

        # Preload the exp activation table set during the DMA lead-in so the
        # first real exp doesn't pay the ~2.7us ACT_TABLE_LOAD.
        warm_sb = persist.tile([1, 32], F32)
        nc.vector.memset(warm_sb, 0.0)
        nc.scalar.activation(out=warm_sb, in_=warm_sb,
                             func=mybir.ActivationFunctionType.Exp)

        # ---------------- Phase 1: K/Q projections ----------------
        # c/w/v stay alive into the attention phase (V projection is emitted
        # inside the first attention block so scores/exp start ASAP).
        cw_pool = stk.enter_context(tc.tile_pool(name="cw", bufs=1))

        w_sb = {}
        for nm, d in (("wkt", wkt_d), ("wqt", wqt_d), ("wvt", wvt_d)):
            w = cw_pool.tile([128, 4, 128], F16, tag=nm)
            nc.sync.dma_start(out=w, in_=d.rearrange("(c p) m -> p c m", p=128))
            w_sb[nm] = w
        c_t = []
        for cc in range(4):
            t = cw_pool.tile([128, N], F16, tag="c_in", bufs=4)
            nc.sync.dma_start(out=t, in_=c_d[ts(cc, 128), :])
            c_t.append(t)
        v_sb = cw_pool.tile([128, N], F16, tag="v_sb")

        # Round-robin the PSUM->SBUF evacuations over Vector and Scalar so
        # neither engine serializes them (GpSimd has no PSUM port).
        _cp_state = {"n": 0}

        def copy_rr(out, in_):
            _cp_state["n"] += 1
            if _cp_state["n"] % 2:
                nc.vector.tensor_copy(out=out, in_=in_)
            else:
                nc.scalar.copy(out=out, in_=in_)

        def project(psum_pool, tag, wname, src, dst):
            w = w_sb[wname]
            for t in range(N // NI):
                ps = psum_pool.tile([128, NI], F32, tag=tag)
                for cc in range(4):
                    nc.tensor.matmul(
                        ps, lhsT=w[:, cc, :], rhs=src[cc][:, ts(t, NI)],
                        start=(cc == 0), stop=(cc == 3),
                    )
                copy_rr(out=dst[:, ts(t, NI)], in_=ps)

        with tc.tile_pool(name="x_in", bufs=1) as x_pool, \
             tc.tile_pool(name="pj_ps", bufs=2, space="PSUM") as pj_ps:
            # x comes in on the gpsimd DMA queue, concurrent with c on sync
            x_t = []
            for cc in range(4):
                t = x_pool.tile([128, N], F16, tag="x_in", bufs=4)
                nc.gpsimd.dma_start(out=t, in_=x_d[ts(cc, 128), :])
                x_t.append(t)

            project(pj_ps, "pj", "wkt", c_t, k_sb)
            project(pj_ps, "pj", "wqt", x_t, q_sb)

        def emit_v_projection(psum_pool, part, nparts):
            # one slice of the V projection + transposes, interleaved between
            # attention blocks of (i=0, p=0) so ScalarE never starves long
            w = w_sb["wvt"]
            nt = N // NI
            for t in range(part * nt // nparts, (part + 1) * nt // nparts):
                ps = psum_pool.tile([128, NI], F32, tag="av",
                                    name=f"vps_{t}")
                for cc in range(4):
                    nc.tensor.matmul(
                        ps, lhsT=w[:, cc, :], rhs=c_t[cc][:, ts(t, NI)],
                        start=(cc == 0), stop=(cc == 3),
                    )
                copy_rr(out=v_sb[:, ts(t, NI)], in_=ps)
            nch = N // NJ
            for ch in range(part * nch // nparts, (part + 1) * nch // nparts):
                tp = psum_pool.tile([128, 128], F16, tag="av",
                                    name=f"vtp_{ch}")
                nc.tensor.transpose(tp, v_sb[:, ts(ch, 128)], ident)
                copy_rr(
                    out=vT_sb[:, ch, :, 0:HEAD_DIM],
                    in_=tp.rearrange("p (h d) -> p h d", h=HEADS_PER_CORE),
                )

        # ---------------- Phase 2: attention ----------------
        # Score PSUM is a stream of 512-wide slots (2 per j-tile: one per
        # head of the active pair), packed 3 per 3-bank psum tile so each
        # exp ACTIVATE covers N=1536 (fewer, cheaper ScalarE instructions).
        SLOTS_PER_TILE = 2
        with tc.tile_pool(name="sc_ps", bufs=2, space="PSUM") as sc_ps, \
             tc.tile_pool(name="av_ps", bufs=2, space="PSUM") as av_ps, \
             tc.tile_pool(name="attn", bufs=1) as at_pool:

            n_i = N // NI   # 8
            n_j = N // NJ   # 32
            n_slots = 2 * n_j  # 64 per (i, p)

            pending_out = []

            def flush_outproj():
                while pending_out:
                    oi, rsb = pending_out.pop(0)
                    for cc in range(4):
                        ps = av_ps.tile([128, NI], F32, tag="av",
                                        name=f"ofps_{oi}_{cc}")
                        nc.tensor.matmul(
                            ps, lhsT=wot_sb[:, ts(cc, 128)], rhs=rsb,
                            start=True, stop=True,
                        )
                        ot = at_pool.tile([128, NI], F32, tag="ot", bufs=4,
                                          name=f"ot_{oi}_{cc}")
                        copy_rr(out=ot, in_=ps)
                        nc.sync.dma_start(
                            out=y_d[ts(cc, 128), ts(oi, NI)], in_=ot)

            for i in range(n_i):
                raw_sb = at_pool.tile([128, NI], F16, tag="raw", bufs=2)
                for p in range(2):
                    exp_ref = [None] * n_slots  # slot -> (es_tile, offset)
                    state = {"ps": None, "es": None, "n": 0, "uid": 0}

                    def flush_exp():
                        if state["ps"] is not None and state["n"] > 0:
                            width = state["n"] * NI
                            eng = EXP_SCHED[(state["uid"] - 1) % len(EXP_SCHED)]
                            if eng == "A":
                                nc.scalar.activation(
                                    out=state["es"][:, 0:width],
                                    in_=state["ps"][:, 0:width],
                                    func=mybir.ActivationFunctionType.Exp,
                                )
                            else:
                                e = nc.vector if eng == "D" else nc.gpsimd
                                e.tensor_scalar(
                                    out=state["es"].bitcast(I16)[:, 0:width],
                                    in0=state["ps"][:, 0:width],
                                    scalar1=EXP_A, scalar2=EXP_B,
                                    op0=mybir.AluOpType.mult,
                                    op1=mybir.AluOpType.add,
                                )
                        state["ps"] = None
                        state["n"] = 0

                    def emit_scores(j0, jn):
                        for j in range(j0, j0 + jn):
                            for hh in range(2):
                                if state["ps"] is None:
                                    state["uid"] += 1
                                    u = state["uid"]
                                    state["ps"] = sc_ps.tile(
                                        [128, SLOTS_PER_TILE * NI], F32,
                                        tag="sc", bufs=3,
                                        name=f"scps_{i}_{p}_{u}")
                                    state["es"] = at_pool.tile(
                                        [128, SLOTS_PER_TILE * NI], F16,
                                        tag="exp", bufs=16,
                                        name=f"es_{i}_{p}_{u}")
                                off = state["n"] * NI
                                base = 64 * p + 32 * hh
                                nc.tensor.matmul(
                                    state["ps"][:, off:off + NI],
                                    lhsT=k_sb[base:base + 32, ts(j, NJ)],
                                    rhs=q_sb[base:base + 32, ts(i, NI)],
                                    start=True, stop=True,
                                    tile_position=(base, 0),
                                )
                                exp_ref[2 * j + hh] = (state["es"], off)
                                state["n"] += 1
                                if state["n"] == SLOTS_PER_TILE:
                                    flush_exp()
                        flush_exp()  # block boundary: never hold a partial

                    def emit_av(j0, jn):
                        for j in range(j0, j0 + jn):
                            for hh in range(2):
                                es, off = exp_ref[2 * j + hh]
                                nc.tensor.matmul(
                                    acc[64 * hh:64 * hh + HEAD_DIM + 1, :],
                                    lhsT=vT_sb[:, j, 2 * p + hh, :],
                                    rhs=es[:, off:off + NI],
                                    start=(j == 0), stop=(j == n_j - 1),
                                    tile_position=(0, 64 * hh),
                                    skip_group_check=True,
                                )

                    blocks = []
                    jj = 0
                    while jj < n_j:
                        jn = min(JBLK, n_j - jj)
                        blocks.append((jj, jn))
                        jj += jn

                    emit_scores(*blocks[0])
                    if p == 0:
                        flush_outproj()
                    if i == 0 and p == 0:
                        emit_v_projection(av_ps, 0, 4)
                    acc = av_ps.tile([128, NI], F32, tag="av")
                    for bi in range(1, len(blocks)):
                        emit_scores(*blocks[bi])
                        if i == 0 and p == 0 and bi <= 3:
                            emit_v_projection(av_ps, bi, 4)
                        emit_av(*blocks[bi - 1])
                    emit_av(*blocks[-1])

                    # normalize: raw[32h:32h+32] = acc_head / l_head
                    for hh in range(2):
                        h = 2 * p + hh
                        lr = at_pool.tile([1, NI], F32, tag="lr", bufs=4)
                        rc = at_pool.tile([1, NI], F32, tag="rc", bufs=4)
                        bc = at_pool.tile([32, NI], F32, tag="bc", bufs=4)
                        nc.vector.tensor_copy(
                            out=lr,
                            in_=acc[64 * hh + HEAD_DIM:64 * hh + HEAD_DIM + 1, :],
                        )
                        nc.vector.reciprocal_approx_fast(out=rc, in_=lr)
                        nc.gpsimd.partition_broadcast(bc, rc)
                        nc.vector.tensor_mul(
                            out=raw_sb[ts(h, 32), :],
                            in0=acc[64 * hh:64 * hh + 32, :],
                            in1=bc,
                        )

                pending_out.append((i, raw_sb))
            flush_outproj()

    nc.compile()
    return nc


_NC_CACHE = None


def _get_nc():
    global _NC_CACHE
    if _NC_CACHE is None:
        _NC_CACHE = build_nc()
    return _NC_CACHE


def _shard_inputs(query, context, Wq, Wk, Wv, Wo):
    query = np.asarray(query, dtype=np.float32)
    context = np.asarray(context, dtype=np.float32)
    Wq = np.asarray(Wq, dtype=np.float32)
    Wk = np.asarray(Wk, dtype=np.float32)
    Wv = np.asarray(Wv, dtype=np.float32)
    Wo = np.asarray(Wo, dtype=np.float32)
    b = query.shape[0]
    in_maps = []
    for core in range(N_CORES):
        bb, p = divmod(core, 2)
        sl = slice(128 * p, 128 * (p + 1))
        in_maps.append({
            "x": np.ascontiguousarray(query[bb].reshape(DIM, N).astype(np.float16)),
            "c": np.ascontiguousarray(context[bb].reshape(DIM, N).astype(np.float16)),
            "wqt": np.ascontiguousarray((Wq[sl, :] * SCALE).T.astype(np.float16)),
            "wkt": np.ascontiguousarray(Wk[sl, :].T.astype(np.float16)),
            "wvt": np.ascontiguousarray(Wv[sl, :].T.astype(np.float16)),
            "wot": np.ascontiguousarray(Wo[:, sl].T.astype(np.float16)),
        })
    return in_maps, b


def _run(inputs, trace=False, **kw):
    in_maps, b = _shard_inputs(**inputs)
    nc = _get_nc()
    res = run_bass_kernel_spmd(nc, in_maps, core_ids=list(range(N_CORES)),
                               trace=trace, **kw)
    outs = []
    for bb in range(b):
        y = res.results[2 * bb]["y"] + res.results[2 * bb + 1]["y"]
        outs.append(y.reshape(DIM, 64, 64))
    return np.stack(outs).astype(np.float32), res


def kernel(**inputs):
    out, _ = _run(inputs)
    return out

